# revision 1
# baseline (speedup 1.0000x reference)
"""Trainium2 Bass kernel for the faithful-reshape causal attention module.

Math (per the reference's raw row-major reshape [B,L,3D] -> [B,H,L,192]):
block (b, h) consumes x rows [128h, 128h+128) of batch b only:
  qkv   = x_blk @ Wqkv                     # [128, 3072]
  q,k,v = qkv.reshape(2048, 192) split     # pseudo-positions l' = 16a + r
  S     = (q @ k^T) / 4, causal over l'
  o     = softmax(S) @ v  -> reshape [128, 1024]
  y_blk = o @ Wo
32 independent blocks; 8 cores x 4 blocks, zero collectives.

On-core layout: everything is computed in (r, a) pseudo-position order.
S is built transposed ([kpos, qpos]) so PV needs no transposes; the causal
mask in this order is per-128-subtile triangular, applied with affine_select
after exp (fill 0.0). Softmax denominators come from a ones-augmented V
(PSUM row 64 of O^T); normalization is a rank-1 PE broadcast of 1/d.
"""
import sys

sys.path.insert(0, '/opt/trn_rl_repo')

import numpy as np

B, L, D = 2, 2048, 1024
H = 16              # heads == blocks per batch
RB = 128            # x rows per block
D3 = 3 * D
NR = 16             # r-groups (192-col chunks per row)
NA = 128            # a positions per r-group
NB = 4              # blocks per core
NCORES = 8
P = 128

_cached = {}


def _build_program():
    import concourse.bass as bass
    import concourse.mybir as mybir
    import concourse.tile as tile
    from concourse.tile import add_dep_helper

    f32 = mybir.dt.float32
    f32r = mybir.dt.float32r
    EXP = mybir.ActivationFunctionType.Exp
    GE = mybir.AluOpType.is_ge

    nc = bass.Bass()
    xs = nc.declare_dram_parameter("xs", [NB, RB, D], f32, isOutput=False)
    wqkv = nc.declare_dram_parameter("wqkv", [D, D3], f32, isOutput=False)
    wo = nc.declare_dram_parameter("wo", [D, D], f32, isOutput=False)
    ys = nc.declare_dram_parameter("ys", [NB, RB, D], f32, isOutput=True)

    with tile.TileContext(nc) as tc:
        with (
            tc.tile_pool(name="const", bufs=1) as constp,
            tc.tile_pool(name="wq", bufs=2) as wqp,
            tc.tile_pool(name="wop", bufs=1) as wop,
            tc.tile_pool(name="xp", bufs=1) as xp,
            tc.tile_pool(name="yo", bufs=1) as yop,
            tc.tile_pool(name="xtp", bufs=1) as xtp,
            tc.tile_pool(name="qkvp", bufs=1) as qkvp,
            tc.tile_pool(name="qkt", bufs=2) as qktp,
            tc.tile_pool(name="ktp", bufs=1) as ktp,
            tc.tile_pool(name="vap", bufs=1) as vap,
            tc.tile_pool(name="pp", bufs=3) as pp,
            tc.tile_pool(name="wl", bufs=2) as wlp,
            tc.tile_pool(name="nrm", bufs=1) as nrmp,
            tc.tile_pool(name="ps", bufs=2, space="PSUM") as psp,
            tc.tile_pool(name="qps", bufs=2, space="PSUM") as qpsp,
            tc.tile_pool(name="ot", bufs=2, space="PSUM") as otp,
        ):
            def absorb_on(eng, *prods):
                # Walrus caps every instruction at ONE sync wait. Emit
                # queue-local nops that sync-depend on each producer; the
                # post-pass elides waits covered by these earlier nops
                # (queue dispatch is in-order, so an earlier wait gates all
                # later instructions in the same queue).
                for p in prods:
                    if p is None:
                        continue
                    n = eng.nop(hint="dep")
                    add_dep_helper(n.ins, p.ins, sync=True)

            def absorb(*prods):
                absorb_on(nc.tensor, *prods)

            ps_readers = []          # per ps-pool alloc: the op that evicts it
            ps_n = [0]
            dma_hs = []

            def ps_tile():
                n = ps_n[0]
                if n >= 2:
                    absorb(ps_readers[n - 2])
                ps_n[0] += 1
                return psp.tile([P, 1024], f32, tag="ps", name="pstile")

            qps_readers = []
            qps_n = [0]

            def qps_tile():
                n = qps_n[0]
                if n >= 2:
                    absorb(qps_readers[n - 2])
                qps_n[0] += 1
                return qpsp.tile([P, 512], f32, tag="qps", name="qpstile")

            ot_readers = []
            ot_n = [0]

            def ot_tile():
                n = ot_n[0]
                if n >= 2:
                    absorb(ot_readers[n - 2])
                ot_n[0] += 1
                return otp.tile([65, 512], f32, tag="ot", name="otq")

            const = constp.tile([P, 128], f32, tag="const")
            ident = const[:, 0:128]
            h_idm = nc.gpsimd.memset(ident, 0.0)
            absorb_on(nc.gpsimd, h_idm)
            nc.gpsimd.affine_select(
                out=ident, in_=ident, compare_op=mybir.AluOpType.not_equal,
                fill=1.0, base=0, pattern=[[-1, 128]], channel_multiplier=1)
            ones_f = constp.tile([P, 144], f32, tag="onesf")
            nc.gpsimd.memset(ones_f[:], 1.0)
            ones16 = ones_f[:, 128:144]
            onesr = constp.tile([1, 128], f32r, tag="onesr")
            h_ones = nc.vector.tensor_copy(onesr[:], ones_f[0:1, 0:128])
            absorb(h_ones)


            stg_hist = []
            # ---- phase A: x load + transpose; stream Wqkv, qkv gemm per block
            xT = xtp.tile([P, NB, 8, P], f32r, tag="xT")
            x_hist = []
            for i in range(NB):
                if len(x_hist) >= 1:
                    absorb_on(nc.gpsimd, *x_hist[-1])
                x_sb = xp.tile([P, D], f32, tag="x")
                h_x = nc.gpsimd.dma_start(x_sb[:], xs[i])
                dma_hs.append(h_x)
                absorb(h_x)
                h_tr = None
                for k in range(8):
                    tp = qps_tile()
                    h_tr = nc.tensor.transpose(
                        tp[:, 0:128], x_sb[:, 128 * k:128 * k + 128], ident)
                    h_e = nc.vector.tensor_copy(xT[:, i, k, :], tp[:, 0:128])
                    qps_readers.append(h_e)
                x_hist.append((h_x, h_tr))

            qkv = qkvp.tile([P, NB, D3], f32, tag="qkv")
            evict_h = {}
            wq_hist = []          # last PE reader of each wq_t
            wq_cast_hist = []     # the DVE cast of each wq_t
            blk = {}
            kt_hist = []
            qkT0 = qktp.tile([P, NR, NA], f32r, tag="qkT", name="qkT0")
            kT0 = ktp.tile([64, NR, NA], f32r, tag="kT", name="kT0")
            va0 = vap.tile([P, NR, 65], f32r, tag="va", name="va0")
            # r-group r of block 0 becomes transposable once qkv cols
            # [192r, 192r+192) exist, i.e. after 256-col chunk (192r+191)//256
            rs_ready = {}
            for r in range(NR):
                rs_ready.setdefault((192 * r + 191) // 256, []).append(r)
            for nch in range(12):
                if len(stg_hist) >= 2:
                    absorb_on(nc.gpsimd, *stg_hist[-2])
                stg = wqp.tile([P, 8, 256], f32, tag="stg")
                h_sd = nc.gpsimd.dma_start(
                    stg[:],
                    wqkv.rearrange("(kc p) n -> p kc n", p=P)[:, :, 256 * nch:256 * nch + 256],
                )
                dma_hs.append(h_sd)
                wq_t = wqp.tile([P, 8, 256], f32r, tag="wq")
                cast_eng = nc.scalar if nch < 6 else nc.vector
                if len(wq_hist) >= 2:
                    absorb_on(cast_eng, wq_hist[-2], wq_cast_hist[-2])
                if len(stg_hist) >= 2:
                    absorb_on(cast_eng, stg_hist[-2][1])
                if nch < 6:
                    h_cast = nc.scalar.copy(wq_t[:], stg[:])
                else:
                    h_cast = nc.vector.tensor_copy(wq_t[:], stg[:])
                stg_hist.append((h_sd, h_cast))
                wq_cast_hist.append(h_cast)
                absorb(h_cast)
                h_mm = None
                for i in range(NB):
                    qp = qps_tile()
                    for k in range(8):
                        h_mm = nc.tensor.matmul(
                            qp[:, 0:256],
                            xT[:, i, k, :],
                            wq_t[:, k, :],
                            start=(k == 0), stop=(k == 7),
                        )
                    h_ev = nc.vector.tensor_copy(
                        qkv[:, i, 256 * nch:256 * nch + 256], qp[:, 0:256])
                    qps_readers.append(h_ev)
                    evict_h[(nch, i)] = h_ev
                wq_hist.append(h_mm)
                # block-0 setup interleaved with streaming: transpose the
                # r-groups whose qkv[:,0,:] columns just completed
                qkv0_rc = qkv[:, 0, :].rearrange("p (r c) -> p r c", r=NR)
                for r in rs_ready.get(nch, []):
                    absorb(evict_h[(nch, 0)])
                    tp = qps_tile()
                    nc.tensor.transpose(
                        tp[:, 0:128], qkv[:, 0, 192 * r:192 * r + 128], ident)
                    h_e = nc.vector.tensor_copy(qkT0[:, r, :], tp[:, 0:128])
                    qps_readers.append(h_e)
                    # per-r operand copies so block-0 strips start at DMA pace
                    h_kt0 = nc.vector.tensor_copy(
                        kT0[:, r:r + 1, :], qkT0[64:128, r:r + 1, :])
                    h_vc0 = nc.vector.tensor_copy(
                        va0[:, r:r + 1, 0:64], qkv0_rc[:, r:r + 1, 128:192])
                    h_vm0 = nc.vector.tensor_copy(
                        va0[:, r:r + 1, 64:65], ones16[:, r:r + 1].unsqueeze(2))
                    absorb(h_kt0, h_vm0)
                if nch == 11:
                    kt_hist.append(h_kt0)
                    absorb(h_vc0, h_vm0)
                    blk[0] = (qkT0, kT0, va0, h_vc0, h_vm0, h_kt0)

            # 0/1 causal mega-strip: window [16-r2+8*h2 : +8] of 128-chunks
            # masks strip (r2, h2): strict tril for r1 < r2, incl for r1 >= r2.
            # Built here (Pool queue reaches it in ~3us, while DMAs stream)
            # so the startup Pool chain before the first DMA trigger is short.
            mega = constp.tile([P, 32, 128], f32, tag="mega", name="mega")
            h_mg0 = nc.gpsimd.memset(mega[:], 1.0)
            absorb_on(nc.gpsimd, h_mg0)
            nc.gpsimd.affine_select(
                out=mega[:, 0:16, :], in_=mega[:, 0:16, :],
                compare_op=GE, fill=0.0, base=-1,
                pattern=[[0, 16], [1, 128]], channel_multiplier=-1)
            h_mg = nc.gpsimd.affine_select(
                out=mega[:, 16:32, :], in_=mega[:, 16:32, :],
                compare_op=GE, fill=0.0, base=0,
                pattern=[[0, 16], [1, 128]], channel_multiplier=-1)
            absorb_on(nc.vector, h_mg)

            wo_sb = wop.tile([P, 8, D], f32r, tag="wo")
            for wc in range(4):
                if len(stg_hist) >= 2:
                    absorb_on(nc.gpsimd, *stg_hist[-2])
                stg = wqp.tile([P, 8, 256], f32, tag="stg")
                h_sd = nc.gpsimd.dma_start(
                    stg[:],
                    wo.rearrange("(kc p) n -> p kc n", p=P)[:, :, 256 * wc:256 * wc + 256],
                )
                dma_hs.append(h_sd)
                if len(stg_hist) >= 2:
                    absorb_on(nc.vector, stg_hist[-2][1])
                h_sc = nc.vector.tensor_copy(wo_sb[:, :, 256 * wc:256 * wc + 256], stg[:])
                stg_hist.append((h_sd, h_sc))

            # ---- phase B: per-block attention + output projection
            h_vm_prev = None
            h_mul_prev = None
            h_pv_prev_block = None
            y_hist = []
            pt_hist = []
            mul_hist = []
            def setup_block(i, hoist=True):
                from contextlib import nullcontext
                ctx = tc.high_priority(offset=2000) if hoist else nullcontext()
                with ctx:
                    _setup_block_body(i)

            def _setup_block_body(i):
                absorb(evict_h[(11, i)],
                       kt_hist[-2] if len(kt_hist) >= 2 else None)
                qkT = qktp.tile([P, NR, NA], f32r, tag="qkT", name="qkT")
                for r in range(NR):
                    tp = qps_tile()
                    nc.tensor.transpose(
                        tp[:, 0:128], qkv[:, i, 192 * r:192 * r + 128], ident)
                    h_e = nc.vector.tensor_copy(qkT[:, r, :], tp[:, 0:128])
                    qps_readers.append(h_e)
                absorb_on(nc.vector, blk[i - 1][3] if i else None,
                          blk[i - 1][4] if i else None)
                kT = ktp.tile([64, NR, NA], f32r, tag="kT", name="kT")
                nc.vector.tensor_copy(kT[:, 0:8, :], qkT[64:128, 0:8, :])
                h_kt = nc.vector.tensor_copy(kT[:, 8:16, :], qkT[64:128, 8:16, :])
                kt_hist.append(h_kt)

                v_aug = vap.tile([P, NR, 65], f32r, tag="va", name="va")
                qkv_rc = qkv[:, i, :].rearrange("p (r c) -> p r c", r=NR)
                nc.vector.tensor_copy(v_aug[:, 0:8, 0:64], qkv_rc[:, 0:8, 128:192])
                h_vc = nc.vector.tensor_copy(
                    v_aug[:, 8:16, 0:64], qkv_rc[:, 8:16, 128:192])
                nc.vector.tensor_copy(v_aug[:, 0:8, 64:65], ones16[:, 0:8].unsqueeze(2))
                h_vm = nc.vector.tensor_copy(
                    v_aug[:, 8:16, 64:65], ones16[:, 8:16].unsqueeze(2))
                absorb(h_vc, h_vm)
                blk[i] = (qkT, kT, v_aug, h_vc, h_vm, h_kt)

            y_hist = []
            pt_hist = []
            mul_hist = []
            h_pv_prev_block = None
            pending_tail = []
            tail_last = {}

            def emit_tail(ti, t_wo_lhsT, t_h_mul):
                yp = ps_tile()
                absorb(t_h_mul)
                for n2 in range(2):
                    for k in range(8):
                        tail_last["womm"] = nc.tensor.matmul(
                            yp[:, 512 * n2:512 * n2 + 512],
                            t_wo_lhsT[:, k, :],
                            wo_sb[:, k, 512 * n2:512 * n2 + 512],
                            start=(k == 0), stop=(k == 7),
                        )
                if len(y_hist) >= 1:
                    absorb_on(nc.gpsimd, *y_hist[-1])
                    absorb_on(nc.vector, *y_hist[-1])
                y_sb = yop.tile([P, D], f32, tag="y")
                h_ye = nc.vector.tensor_copy(y_sb[:], yp[:])
                tail_last["ye"] = h_ye
                ps_readers.append(h_ye)
                h_yd = nc.gpsimd.dma_start(ys[ti], y_sb[:])
                dma_hs.append(h_yd)
                y_hist.append((h_ye, h_yd))

            for i in range(NB):
                if i + 1 < NB:
                    setup_block(i + 1)
                qkT, kT, v_aug = blk[i][0], blk[i][1], blk[i][2]
                wo_lhsT = wlp.tile([P, 8, P], f32r, tag="wl")
                h_mul = None
                if pending_tail:
                    emit_tail(*pending_tail.pop(0))
                for h2 in range(2):
                    absorb(blk[i][5], blk[i][4])
                    otq = [ot_tile(), ot_tile()]
                    for r2 in range(NR):
                        stt = ps_tile()
                        for half in range(2):
                            nc.tensor.matmul(
                                stt[:, 512 * half:512 * half + 512],
                                kT[:, r2, :],
                                qkT[0:64, 8 * h2 + 4 * half:8 * h2 + 4 * half + 4, :]
                                .rearrange("c r a -> c (r a)"),
                                start=True, stop=True,
                            )
                        if len(pt_hist) >= 3:
                            h_exp3, h_aff3, h_pv3 = pt_hist[-3]
                            absorb_on(nc.scalar, h_exp3, h_aff3)
                            absorb_on(nc.gpsimd, h_aff3, h_pv3)
                            absorb_on(nc.vector, h_aff3, h_pv3)
                        pt = pp.tile([P, 1024], f32r, tag="p")
                        h_exp = nc.scalar.activation(pt[:], stt[:], EXP, scale=0.25)
                        ps_readers.append(h_exp)
                        absorb_on(nc.gpsimd, h_exp)
                        # causal mask: keep col a1 of subtile r1 iff a1-a2 >= delta
                        # delta = 1 (strict) for r1 < r2, 0 (incl) for r1 >= r2.
                        # Alternate engines so neither Pool nor DVE bottlenecks.
                        c = min(max(r2 - 8 * h2, 0), 8)
                        if r2 % 2 == 1:
                            absorb_on(nc.vector, h_exp)
                            h_aff = nc.vector.tensor_mul(
                                pt[:],
                                pt[:].bitcast(f32),
                                mega[:, 16 - r2 + 8 * h2:24 - r2 + 8 * h2, :]
                                .rearrange("p j a -> p (j a)"),
                            )
                        else:
                            h_aff = h_exp
                            if c > 0:
                                h_aff = nc.gpsimd.affine_select(
                                    out=pt[:, 0:128 * c].rearrange("p (j a) -> p j a", j=c),
                                    in_=pt[:, 0:128 * c].rearrange("p (j a) -> p j a", j=c),
                                    compare_op=GE, fill=0.0, base=-1,
                                    pattern=[[0, c], [1, 128]], channel_multiplier=-1,
                                )
                            if c < 8:
                                h_aff = nc.gpsimd.affine_select(
                                    out=pt[:, 128 * c:1024].rearrange("p (j a) -> p j a", j=8 - c),
                                    in_=pt[:, 128 * c:1024].rearrange("p (j a) -> p j a", j=8 - c),
                                    compare_op=GE, fill=0.0, base=0,
                                    pattern=[[0, 8 - c], [1, 128]], channel_multiplier=-1,
                                )
                        absorb(h_exp, h_aff)
                        h_pv = None
                        for half in range(2):
                            h_pv = nc.tensor.matmul(
                                otq[half][:, :],
                                v_aug[:, r2, :],
                                pt[:, 512 * half:512 * half + 512],
                                start=(r2 == 0), stop=(r2 == NR - 1),
                            )
                        pt_hist.append((h_exp, h_aff, h_pv))
                        h_pv_prev_block = h_pv

                    # normalize this h2's two qpos quarters while later strips run
                    for q in range(2):
                        rcp = nrmp.tile([1, 512], f32r, tag="rcp")
                        with nc.allow_low_precision(reason="f32r rounding of 1/d"):
                            h_rcp = nc.vector.reciprocal(rcp[:], otq[q][64:65, :])
                        bc = qps_tile()
                        absorb(h_rcp)
                        nc.tensor.matmul(
                            bc[:, 0:512], onesr[:], rcp[:, 0:512],
                            start=True, stop=True,
                        )
                        if mul_hist:
                            absorb_on(nc.vector, mul_hist[-1])
                        bc_sb = nrmp.tile([64, 512], f32, tag="bc")
                        h_bcc = nc.vector.tensor_copy(bc_sb[:], bc[0:64, 0:512])
                        qps_readers.append(h_bcc)
                        ot_rc = otq[q][0:64, :].rearrange("p (j a) -> p j a", j=4)
                        bc_rc = bc_sb[:].rearrange("p (j a) -> p j a", j=4)
                        ch = 4 * h2 + 2 * q
                        nc.vector.tensor_mul(
                            wo_lhsT[0:64, ch:ch + 2, :],
                            ot_rc[:, 0:4:2, :], bc_rc[:, 0:4:2, :])
                        h_mul = nc.vector.tensor_mul(
                            wo_lhsT[64:128, ch:ch + 2, :],
                            ot_rc[:, 1:4:2, :], bc_rc[:, 1:4:2, :])
                        mul_hist.append(h_mul)
                        ot_readers.append(h_mul)
                pending_tail.append((i, wo_lhsT, h_mul))

            while pending_tail:
                emit_tail(*pending_tail.pop(0))

            # absorb the kernel-tail drain's dependencies onto SP nops
            absorb_on(nc.sync, *dma_hs)
            absorb_on(nc.sync, h_aff, h_exp, tail_last["ye"], tail_last["womm"],
                      h_mul, h_pv)

    return nc


def _elide_covered_waits(nc):
    """Walrus rejects >1 sync wait per instruction. Each queue's sequencer
    processes waits in dispatch order, so a wait already issued earlier in
    the same queue gates every later instruction in that queue. Drop waits
    that an earlier same-queue instruction (incl. absorber nops) covers."""
    observed = {}   # engine -> {sem_id: max waited value}
    leftover = []
    for inst in nc.all_instructions():
        si = inst.sync_info
        if si is None:
            continue
        if type(inst).__name__ in ("InstEventSemaphore", "InstTrigger"):
            continue  # barrier-protocol handshakes, not data waits
        eng = str(inst.engine)
        obs = observed.setdefault(eng, {})
        ow = list(si.on_wait or [])
        keep = []
        for w in ow:
            if obs.get(w.id, -1) >= w.wait_value:
                continue
            keep.append(w)
            obs[w.id] = max(obs.get(w.id, -1), w.wait_value)
        if len(keep) != len(ow):
            si.on_wait = keep
            inst.sync_info = si
        if len(keep) > 1:
            leftover.append((inst.name, type(inst).__name__, eng,
                             [(w.ant_name, w.wait_value) for w in keep]))
    if leftover:
        import logging
        logging.warning("multi-wait instructions remain: %s", leftover[:12])


def _get_program():
    if "nc" not in _cached:
        nc = _build_program()
        _elide_covered_waits(nc)
        _cached["nc"] = nc
    return _cached["nc"]


def kernel(x=None, mask=None, Wqkv=None, Wo=None, **_ignored):
    """Full inputs -> full output. mask is ignored (guaranteed causal tril)."""
    from concourse.bass_utils import run_bass_kernel_spmd

    x = np.ascontiguousarray(np.asarray(x, dtype=np.float32))
    Wqkv = np.ascontiguousarray(np.asarray(Wqkv, dtype=np.float32))
    Wo = np.ascontiguousarray(np.asarray(Wo, dtype=np.float32))

    nc = _get_program()
    in_maps = []
    for c in range(NCORES):
        shards = []
        for g in range(NB * c, NB * c + NB):
            b, h = divmod(g, H)
            shards.append(x[b, RB * h:RB * h + RB, :])
        in_maps.append({
            "xs": np.ascontiguousarray(np.stack(shards, axis=0)),
            "wqkv": Wqkv,
            "wo": Wo,
        })

    res = run_bass_kernel_spmd(nc, in_maps, core_ids=list(range(NCORES)))
    y = np.empty((B, L, D), dtype=np.float32)
    for c in range(NCORES):
        ysc = res.results[c]["ys"]
        for idx, g in enumerate(range(NB * c, NB * c + NB)):
            b, h = divmod(g, H)
            y[b, RB * h:RB * h + RB, :] = ysc[idx]
    return y



# revision 23
# speedup vs baseline: 1.1817x; 1.1817x over previous
"""Trainium2 Bass kernel for the faithful-reshape causal attention module.

Math (per the reference's raw row-major reshape [B,L,3D] -> [B,H,L,192]):
block (b, h) consumes x rows [128h, 128h+128) of batch b only:
  qkv   = x_blk @ Wqkv                     # [128, 3072]
  q,k,v = qkv.reshape(2048, 192) split     # pseudo-positions m = 16a + r
  S     = (q @ k^T) / 4, causal over m
  o     = softmax(S) @ v  -> reshape [128, 1024]
  y_blk = o @ Wo
32 independent blocks; 8 cores x 4 blocks, zero collectives.

v2 (causal-skip restructure): q^T/k^T/v^T are materialized in TRUE
pseudo-position order (m = 16a + r), so S^T [kpos, qpos] decomposes into
16 kpos-strips x 4 qpos-chunks of which only the block-lower-triangular
half is live (53% of tiles). Off-diagonal live tiles need no mask at
all; the 16 diagonal 128x128 tiles get a plain tril affine_select.
S/PV run in bf16 (same modeled PE rate as f32r, half the DVE/SBUF
traffic); the qkv and Wo gemms stay f32r with weights DMA'd via dtype
bitcast (zero cast traffic). q^T/k^T come from SP-issued DMA-transposes
(bf16) + one strided re-layout copy each on Pool; v^T via PE transposes
with strided m-order evictions, then per-strip back-transposes into
[kpos, c] form for PV.
"""
import sys

sys.path.insert(0, '/opt/trn_rl_repo')

import numpy as np

B, L, D = 2, 2048, 1024
H = 16              # heads == blocks per batch
RB = 128            # x rows per block
D3 = 3 * D
NR = 16             # r-groups (192-col chunks per row)
NB = 4              # blocks per core
NCORES = 8
P = 128
NCH = 6             # Wqkv 512-col streaming chunks
NC = 4              # qpos chunks of 512 per block
NT = 16             # kpos strips of 128 per block

_cached = {}


def _build_program():
    import concourse.bass as bass
    import concourse.mybir as mybir
    import concourse.tile as tile
    from concourse.tile import add_dep_helper

    f32 = mybir.dt.float32
    f32r = mybir.dt.float32r
    bf16 = mybir.dt.bfloat16
    EXP = mybir.ActivationFunctionType.Exp
    GE = mybir.AluOpType.is_ge

    nc = bass.Bass()
    xs = nc.declare_dram_parameter("xs", [NB, RB, D], f32, isOutput=False)
    wqkv = nc.declare_dram_parameter("wqkv", [D, D3], f32, isOutput=False)
    wo = nc.declare_dram_parameter("wo", [D, D], f32, isOutput=False)
    ys = nc.declare_dram_parameter("ys", [NB, RB, D], f32, isOutput=True)
    DBG = _cached.get("debug", False)
    if DBG:
        d_qkv = nc.declare_dram_parameter("d_qkv", [P, D3], f32, isOutput=True)
        d_qT = nc.declare_dram_parameter("d_qT", [64, L], f32, isOutput=True)
        d_kT = nc.declare_dram_parameter("d_kT", [64, L], f32, isOutput=True)
        d_vT = nc.declare_dram_parameter("d_vT", [64, L], f32, isOutput=True)
        d_va = nc.declare_dram_parameter("d_va", [P, NT * 65], f32, isOutput=True)
        d_pt = nc.declare_dram_parameter("d_pt", [P, 1024], f32, isOutput=True)
        d_osb = nc.declare_dram_parameter("d_osb", [64, 512], f32, isOutput=True)
        d_qkvT = nc.declare_dram_parameter("d_qkvT", [P, 3072], f32, isOutput=True)

    from contextlib import ExitStack
    with tile.TileContext(nc) as tc:
        with ExitStack() as _stk:
            def _pool(**kw):
                return _stk.enter_context(tc.tile_pool(**kw))

            constp = _pool(name="const", bufs=1)
            wqp = _pool(name="wq", bufs=2)
            wop = _pool(name="wop", bufs=1)
            xp = _pool(name="xp", bufs=2)
            xtp = _pool(name="xtp", bufs=1)
            qkvp = _pool(name="qkvp", bufs=1)
            qkcrap = _pool(name="qkcra", bufs=2)
            qtmp = _pool(name="qtm", bufs=2)
            ktmp = _pool(name="ktm", bufs=2)
            vtmp = _pool(name="vtm", bufs=2)
            vaugp = _pool(name="vaug", bufs=2)
            wlp = _pool(name="wl", bufs=2)
            ptp = _pool(name="pt", bufs=3)
            nrmp = _pool(name="nrm", bufs=2)
            yop = _pool(name="yo", bufs=2)
            qpsp = _pool(name="qps", bufs=2, space="PSUM")
            sttp = _pool(name="stt", bufs=2, space="PSUM")
            otqp = _pool(name="otq", bufs=2, space="PSUM")
            _pend_nops = []

            def absorb_on(eng, *prods):
                # Walrus caps every instruction at ONE sync wait. Emit
                # queue-local nops that sync-depend on each producer; the
                # post-pass elides waits covered by these earlier nops.
                for p in prods:
                    if p is None:
                        continue
                    n = eng.nop(hint="dep")
                    add_dep_helper(n.ins, p.ins, sync=True)
                    _pend_nops.append(n)

            def pin(h):
                # keep absorber nops scheduled before their instruction
                while _pend_nops:
                    n = _pend_nops.pop()
                    add_dep_helper(h.ins, n.ins, sync=False)
                return h

            dma_hs = []

            def dma(eng, dst, src, *deps):
                _pend_nops.clear()
                absorb_on(eng, *deps)
                absorb_on(eng, *dma_hs[-4:])
                h = pin(eng.dma_start(dst, src))
                dma_hs.append(h)
                return h

            # shared PSUM ring [128, 512] f32 for qkv-gemm outs, packed
            # transposes, bc broadcasts and wo-gemm outs
            qps_readers = []
            qps_n = [0]

            def qps_tile():
                n = qps_n[0]
                if n >= 2:
                    absorb_on(nc.tensor, qps_readers[n - 2])
                qps_n[0] += 1
                return qpsp.tile([P, 512], f32, tag="qps", name="qpstile")

            # ---- constants
            identb = constp.tile([P, 128], bf16, tag="identb")
            h_idm = nc.gpsimd.memset(identb[:], 0.0)
            absorb_on(nc.gpsimd, h_idm)
            h_idb = nc.gpsimd.affine_select(
                out=identb[:], in_=identb[:],
                compare_op=mybir.AluOpType.not_equal,
                fill=1.0, base=0, pattern=[[-1, 128]], channel_multiplier=1)
            identr_f = constp.tile([P, 128], f32, tag="identr")
            h_idm2 = nc.gpsimd.memset(identr_f[:], 0.0)
            absorb_on(nc.gpsimd, h_idm2)
            h_idr = nc.gpsimd.affine_select(
                out=identr_f[:], in_=identr_f[:],
                compare_op=mybir.AluOpType.not_equal,
                fill=1.0, base=0, pattern=[[-1, 128]], channel_multiplier=1)
            identr = identr_f[:]
            ones_f = constp.tile([1, 128], f32, tag="onesf")
            h_of = nc.gpsimd.memset(ones_f[:], 1.0)
            onesr = constp.tile([1, 128], f32r, tag="onesr")
            absorb_on(nc.vector, h_of)
            h_ones = nc.vector.tensor_copy(onesr[:], ones_f[:])

            # ---- phase A: x load + f32r transpose -> xT
            xT = xtp.tile([P, NB, 8, P], f32r, tag="xT")
            x_ev = []           # per block: last xT eviction handle (DVE)
            x_tr = []           # per block: last x transpose handle (PE)
            h_xev = None
            for i in range(NB):
                x_sb = xp.tile([P, D], f32, tag="x")
                h_x = dma(nc.gpsimd, x_sb[:], xs[i],
                          x_tr[i - 2] if i >= 2 else None)
                absorb_on(nc.tensor, h_x, h_idr if i == 0 else None)
                for g in range(2):
                    tp = qps_tile()
                    h_tr = None
                    for j in range(4):
                        h_tr = nc.tensor.transpose(
                            tp[:, 128 * j:128 * j + 128],
                            x_sb[:, 512 * g + 128 * j:512 * g + 128 * j + 128],
                            identr)
                    absorb_on(nc.vector, h_tr)
                    h_xev = nc.vector.tensor_copy(
                        xT[:, i, 4 * g:4 * g + 4, :],
                        tp[:].rearrange("p (k a) -> p k a", k=4))
                    qps_readers.append(h_xev)
                x_ev.append(h_xev)
                x_tr.append(h_tr)

            # ---- phase A: stream Wqkv (f32r bitcast, no casts), qkv gemms,
            # and per-block span DMA-transposes (3 per block, interleaved)
            qkv = qkvp.tile([P, NB, D3], bf16, tag="qkv")
            qkvT = qkvp.tile([P, NB, 24, P], bf16, tag="qkvT")
            evict_h = {}        # (nch, i) -> eviction handle
            span_h = {}         # (i, w) -> dma-transpose handle
            wq_hist = []
            for nch in range(NCH):
                wq_t = wqp.tile([P, 8, 512], f32r, tag="wq")
                h_wd = dma(
                    nc.gpsimd, wq_t[:],
                    wqkv.rearrange("(kc p) n -> p kc n", p=P)
                    [:, :, 512 * nch:512 * nch + 512].bitcast(f32r),
                    wq_hist[-2] if len(wq_hist) >= 2 else None)
                h_mm = None
                for i in range(NB):
                    qp = qps_tile()
                    absorb_on(nc.tensor, h_wd if i == 0 else None,
                              x_ev[i] if nch == 0 else None)
                    for k in range(8):
                        h_mm = nc.tensor.matmul(
                            qp[:, 0:512],
                            xT[:, i, k, :],
                            wq_t[:, k, :],
                            start=(k == 0), stop=(k == 7),
                        )
                    absorb_on(nc.vector, h_mm)
                    h_ev = nc.vector.tensor_copy(
                        qkv[:, i, 512 * nch:512 * nch + 512], qp[:, 0:512])
                    qps_readers.append(h_ev)
                    evict_h[(nch, i)] = h_ev
                    if nch % 2 == 1:
                        w = nch // 2
                        _pend_nops.clear()
                        absorb_on(nc.sync, h_ev, *dma_hs[-12:])
                        h_sp = pin(nc.sync.dma_start_transpose(
                            qkvT[:, i, 8 * w:8 * w + 8, :],
                            qkv[:, i, 1024 * w:1024 * w + 1024]))
                        dma_hs.append(h_sp)
                        span_h[(i, w)] = h_sp
                wq_hist.append(h_mm)

            # ---- phase A: Wo resident (f32r bitcast)
            wo_sb = wop.tile([P, 8, D], f32r, tag="wo")
            h_wo_dma = []
            for wc in range(2):
                h = dma(
                    nc.gpsimd,
                    wo_sb[:, :, 512 * wc:512 * wc + 512],
                    wo.rearrange("(kc p) n -> p kc n", p=P)
                    [:, :, 512 * wc:512 * wc + 512].bitcast(f32r),
                    h_wo_dma[0] if wc == 1 else None)
                h_wo_dma.append(h)

            # ---- per-block setup pieces (emitted interleaved)
            def strip_width(t, c):
                j = t - 4 * c
                return 512 if j < 0 else 512 - 128 * j

            blk = {}            # i -> dict of tiles/handles
            pt_ring = []        # (exp, last_pv, last_writer) per pt use
            pt_n = [0]

            # six m-order re-layout copies per block, from the 128-col slot
            # decomposition: global col g = 128*S + p; q/k/v of r-group r sit
            # at (partition base, slot offset mod 3) per r parity.
            #   name: (dst, src_pbase, slot_off, r0, engine)
            REL = [
                ('vT', 0, 1, 0), ('vT', 64, 2, 1),
                ('kT', 64, 0, 0), ('kT', 0, 2, 1),
                ('qT', 0, 0, 0), ('qT', 64, 1, 1),
            ]

            def war_pe():
                return pt_ring[-1][1] if pt_ring else None

            def emit_relayout(i, idx):
                b = blk.setdefault(i, {})
                if 'qT' not in b:
                    b['qT'] = qtmp.tile([64, L], bf16, tag="qT", name="qTm")
                    b['kT'] = ktmp.tile([64, L], bf16, tag="kT", name="kTm")
                    b['vT'] = vtmp.tile([64, L], bf16, tag="vT", name="vTm")
                    b['rel'] = {}
                name, pbase, off, r0 = REL[idx]
                eng = nc.gpsimd if name == 'kT' else nc.vector
                absorb_on(eng, span_h[(i, 0)], span_h[(i, 1)],
                          span_h[(i, 2)],
                          war_pe() if i >= 2 else None)
                h = eng.tensor_copy(
                    b[name][:].rearrange("p (a r) -> p a r", r=NR)
                    [:, :, r0:NR:2],
                    qkvT[pbase:pbase + 64, i, off:24:3, :]
                    .rearrange("p u a -> p a u"))
                b['rel'][(name, r0)] = h
                return h

            def emit_vaug(i, t_lo, t_hi):
                b = blk[i]
                if 'vaug' not in b:
                    b['vaug'] = vaugp.tile([P, NT, 65], bf16, tag="vaug", name="vaug")
                    # ones live only in col 64, which later blocks' evictions
                    # never touch: initialize the two pool slots once
                    if i < 2:
                        b['h_vm'] = nc.gpsimd.memset(b['vaug'][:], 1.0)
                    else:
                        b['h_vm'] = None
                    b['h_bt_ev'] = [None] * NT
                for t in range(t_lo, t_hi):
                    tp = qps_tile()
                    tpb = tp[:].bitcast(bf16)
                    absorb_on(nc.tensor,
                              b['rel'][('vT', 0)] if t == t_lo else None,
                              b['rel'][('vT', 1)] if t == t_lo else None)
                    h_bt = nc.tensor.transpose(
                        tpb[:, 0:64],
                        b['vT'][:, 128 * t:128 * t + 128],
                        identb[0:64, 0:64])
                    absorb_on(nc.vector, h_bt,
                              b['h_vm'] if t == t_lo else None)
                    h_be = nc.vector.tensor_copy(
                        b['vaug'][:, t, 0:64], tpb[:, 0:64])
                    qps_readers.append(h_be)
                    b['h_bt_ev'][t] = h_be
                    b['h_bt'] = h_bt

            # ---- attention per block, c-major with strip pairs
            stt_ring = []       # exp handle per stt tile use
            stt_n = [0]
            otq_readers = []    # per otq alloc: a normalization reader
            otq_n = [0]
            mul_hist = []
            y_hist = []
            pending_tail = []
            tail_last = {}

            def emit_tail(ti, t_wo_lhsT, t_h_mul):
                absorb_on(nc.tensor, t_h_mul,
                          h_wo_dma[1] if ti == 0 else None)
                y_sb = yop.tile([P, D], f32, tag="y")
                h_ye = None
                for n2 in range(2):
                    yp = qps_tile()
                    for k in range(8):
                        tail_last["womm"] = nc.tensor.matmul(
                            yp[:, 0:512],
                            t_wo_lhsT[:, k, :],
                            wo_sb[:, k, 512 * n2:512 * n2 + 512],
                            start=(k == 0), stop=(k == 7),
                        )
                    absorb_on(nc.vector, tail_last["womm"],
                              y_hist[-1][1] if (n2 == 0 and y_hist) else None)
                    h_ye = nc.vector.tensor_copy(
                        y_sb[:, 512 * n2:512 * n2 + 512], yp[:, 0:512])
                    tail_last["ye"] = h_ye
                    qps_readers.append(h_ye)
                h_yd = dma(nc.gpsimd, ys[ti], y_sb[:], h_ye)
                y_hist.append((h_ye, h_yd))

            # block 0 (and block 1 vT) setup before the attention loop
            for idx in range(6):
                emit_relayout(0, idx)
            emit_vaug(0, 0, NT)

            def tap(dst, src_ap, dep):
                np_ = src_ap.shape[0]
                nf = src_ap.free_size()
                flat_src = src_ap.rearrange(
                    " ".join(["p"] + [f"f{k}" for k in range(len(src_ap.shape) - 1)])
                    + " -> p (" + " ".join(f"f{k}" for k in range(len(src_ap.shape) - 1)) + ")")                     if len(src_ap.shape) > 2 else src_ap
                for o in range(0, nf, 1024):
                    wdt = min(1024, nf - o)
                    t_sb = yop.tile([P, D], f32, tag="y", name="tapbuf")
                    _pend_nops.clear()
                    absorb_on(nc.vector, dep, *dma_hs[-4:])
                    hh = pin(nc.vector.tensor_copy(
                        t_sb[0:np_, 0:wdt], flat_src[:, o:o + wdt]))
                    dma(nc.gpsimd, dst[:, o:o + wdt] if nf > 1024 else dst[0:np_, 0:nf],
                        t_sb[0:np_, 0:wdt], hh)

            h_exp = h_pv = h_mul = None
            for i in range(NB):
                b = blk[i]
                qT, kT, v_aug = b['qT'], b['kT'], b['vaug']
                h_bt_ev = b['h_bt_ev']
                wo_lhsT = wlp.tile([P, 8, P], f32r, tag="wl")
                if pending_tail:
                    emit_tail(*pending_tail.pop(0))
                absorb_on(nc.tensor, b['rel'][('qT', 0)],
                          b['rel'][('qT', 1)], b['rel'][('kT', 0)],
                          b['rel'][('kT', 1)])
                if DBG and i == 0:
                    tap(d_qkv, qkv[:, 0, :], evict_h[(5, 0)])
                    tap(d_qkvT, qkvT[:, 0, :, :].rearrange("p s a -> p (s a)"),
                        span_h[(0, 2)])
                    tap(d_qT, b['qT'][:], b['rel'][('qT', 1)])
                    tap(d_kT, b['kT'][:], b['rel'][('kT', 1)])
                    tap(d_vT, b['vT'][:], b['rel'][('vT', 1)])
                    tap(d_va, b['vaug'][:].rearrange("p t c -> p (t c)"),
                        b['h_bt_ev'][NT - 1])
                for c in range(NC):
                    # interleave next block's setup at chunk boundaries
                    if i + 1 < NB:
                        if c == 0:
                            emit_relayout(i + 1, 0)
                            emit_relayout(i + 1, 1)
                        elif c == 1:
                            emit_relayout(i + 1, 2)
                            emit_relayout(i + 1, 3)
                            emit_vaug(i + 1, 0, 8)
                        elif c == 2:
                            emit_relayout(i + 1, 4)
                            emit_relayout(i + 1, 5)
                            emit_vaug(i + 1, 8, NT)
                    cm0 = 512 * c
                    tmax = 4 * c + 3
                    if otq_n[0] >= 2:
                        absorb_on(nc.tensor, otq_readers[otq_n[0] - 2])
                    otq_n[0] += 1
                    otq = otqp.tile([65, 512], f32, tag="otq", name="otq")
                    for t0 in range(0, tmax + 1, 2):
                        pair = [t for t in (t0, t0 + 1) if t <= tmax]
                        widths = [strip_width(t, c) for t in pair]
                        # S matmuls for the pair into one stt tile
                        if stt_n[0] >= 2:
                            absorb_on(nc.tensor, stt_ring[stt_n[0] - 2])
                        stt_n[0] += 1
                        stt = sttp.tile([P, 1024], f32, tag="stt")
                        off = 0
                        offs = []
                        h_s = None
                        for t, w in zip(pair, widths):
                            absorb_on(nc.tensor,
                                      h_bt_ev[t] if c == t // 4 else None)
                            h_s = nc.tensor.matmul(
                                stt[:, off:off + w],
                                kT[:, 128 * t:128 * t + 128],
                                qT[:, cm0 + 512 - w:cm0 + 512],
                                start=True, stop=True,
                            )
                            offs.append(off)
                            off += w
                        # one exp over the pair
                        if pt_n[0] >= 3:
                            absorb_on(nc.scalar, pt_ring[pt_n[0] - 3][1],
                                      pt_ring[pt_n[0] - 3][2])
                        pt = ptp.tile([P, 1024], bf16, tag="pt")
                        pt_n[0] += 1
                        absorb_on(nc.scalar, h_s)
                        h_exp = nc.scalar.activation(
                            pt[:, 0:off], stt[:, 0:off], EXP, scale=0.25)
                        stt_ring.append(h_exp)
                        # masks on diagonal strips (t in [4c, 4c+3])
                        h_mask = {}
                        h_lastw = h_exp
                        for t, w, o in zip(pair, widths, offs):
                            if t >= 4 * c:
                                absorb_on(nc.gpsimd, h_exp)
                                h_mask[t] = nc.gpsimd.affine_select(
                                    out=pt[:, o:o + 128],
                                    in_=pt[:, o:o + 128],
                                    compare_op=GE, fill=0.0, base=0,
                                    pattern=[[1, 128]], channel_multiplier=-1)
                                h_lastw = h_mask[t]
                        # PV accumulation
                        for t, w, o in zip(pair, widths, offs):
                            absorb_on(nc.tensor, h_mask.get(t, h_exp))
                            h_pv = nc.tensor.matmul(
                                otq[:, 512 - w:512],
                                v_aug[:, t, :],
                                pt[:, o:o + w],
                                start=(t == 0), stop=(t == tmax),
                            )
                        pt_ring.append((h_exp, h_pv, h_lastw))
                        if DBG and i == 0 and c == 0 and t0 == 0:
                            tap(d_pt, pt[:, 0:off], h_lastw)
                    # normalization + wo_lhsT build for this chunk
                    rcp = nrmp.tile([1, 512], f32r, tag="rcp")
                    absorb_on(nc.vector, h_pv)
                    with nc.allow_low_precision(reason="f32r rounding of 1/d"):
                        h_rcp = nc.vector.reciprocal(rcp[:], otq[64:65, :])
                    o_sb = nrmp.tile([64, 512], bf16, tag="osb")
                    h_osb = nc.vector.tensor_copy(o_sb[:], otq[0:64, :])
                    otq_readers.append(h_osb)
                    if DBG and i == 0 and c == 0:
                        tap(d_osb, o_sb[:], h_osb)
                    bc = qps_tile()
                    absorb_on(nc.tensor, h_rcp)
                    h_bc = nc.tensor.matmul(
                        bc[:, 0:512], onesr[:], rcp[:, 0:512],
                        start=True, stop=True,
                    )
                    absorb_on(nc.vector, h_bc)
                    for par in range(2):
                        src = o_sb[:, par:512:2].rearrange(
                            "p (a ch) -> p ch a", ch=8)
                        bsr = bc[0:64, par:512:2].rearrange(
                            "p (a ch) -> p ch a", ch=8)
                        h_mul = nc.vector.tensor_mul(
                            wo_lhsT[64 * par:64 * par + 64, :,
                                    32 * c:32 * c + 32],
                            src, bsr)
                        mul_hist.append(h_mul)
                        qps_readers.append(h_mul)
                pending_tail.append((i, wo_lhsT, h_mul))

            while pending_tail:
                emit_tail(*pending_tail.pop(0))

            # absorb the kernel-tail drain's dependencies onto SP nops
            absorb_on(nc.sync, *dma_hs)
            absorb_on(nc.sync, tail_last["ye"], tail_last["womm"],
                      h_mul, h_pv, h_exp, h_ones)

    return nc


def _elide_covered_waits(nc):
    """Walrus rejects >1 sync wait per instruction. Each queue's sequencer
    processes waits in dispatch order, so a wait already issued earlier in
    the same queue gates every later instruction in that queue. Drop waits
    that an earlier same-queue instruction (incl. absorber nops) covers."""
    observed = {}   # engine -> {sem_id: max waited value}
    leftover = []
    for inst in nc.all_instructions():
        si = inst.sync_info
        if si is None:
            continue
        if type(inst).__name__ in ("InstEventSemaphore", "InstTrigger"):
            continue  # barrier-protocol handshakes, not data waits
        eng = str(inst.engine)
        short = eng.split(".")[-1]
        obs = observed.setdefault(eng, {})
        ow = list(si.on_wait or [])
        keep = []
        for w in ow:
            if obs.get(w.id, -1) >= w.wait_value:
                continue
            if w.ant_name.startswith(short + "_"):
                # wait on this engine's own completion counter: satisfied
                # by in-order execution of the same queue
                obs[w.id] = max(obs.get(w.id, -1), w.wait_value)
                continue
            keep.append(w)
            obs[w.id] = max(obs.get(w.id, -1), w.wait_value)
        if len(keep) != len(ow):
            si.on_wait = keep
            inst.sync_info = si
        if len(keep) > 1:
            leftover.append((inst.name, type(inst).__name__, eng,
                             [(w.ant_name, w.wait_value) for w in keep]))
    if leftover:
        import logging
        logging.warning("multi-wait instructions remain: %s", leftover[:12])


def _get_program():
    if "nc" not in _cached:
        nc = _build_program()
        _elide_covered_waits(nc)
        _cached["nc"] = nc
    return _cached["nc"]


def kernel(x=None, mask=None, Wqkv=None, Wo=None, **_ignored):
    """Full inputs -> full output. mask is ignored (guaranteed causal tril)."""
    from concourse.bass_utils import run_bass_kernel_spmd

    x = np.ascontiguousarray(np.asarray(x, dtype=np.float32))
    Wqkv = np.ascontiguousarray(np.asarray(Wqkv, dtype=np.float32))
    Wo = np.ascontiguousarray(np.asarray(Wo, dtype=np.float32))

    nc = _get_program()
    in_maps = []
    for c in range(NCORES):
        shards = []
        for g in range(NB * c, NB * c + NB):
            b, h = divmod(g, H)
            shards.append(x[b, RB * h:RB * h + RB, :])
        in_maps.append({
            "xs": np.ascontiguousarray(np.stack(shards, axis=0)),
            "wqkv": Wqkv,
            "wo": Wo,
        })

    res = run_bass_kernel_spmd(nc, in_maps, core_ids=list(range(NCORES)))
    y = np.empty((B, L, D), dtype=np.float32)
    for c in range(NCORES):
        ysc = res.results[c]["ys"]
        for idx, g in enumerate(range(NB * c, NB * c + NB)):
            b, h = divmod(g, H)
            y[b, RB * h:RB * h + RB, :] = ysc[idx]
    return y


# revision 34
# speedup vs baseline: 1.4294x; 1.2096x over previous
"""Trainium2 Bass kernel for the faithful-reshape causal attention module.

Math (per the reference's raw row-major reshape [B,L,3D] -> [B,H,L,192]):
block (b, h) consumes x rows [128h, 128h+128) of batch b only:
  qkv   = x_blk @ Wqkv                     # [128, 3072]
  q,k,v = qkv.reshape(2048, 192) split     # pseudo-positions m = 16a + r
  S     = (q @ k^T) / 4, causal over m
  o     = softmax(S) @ v  -> reshape [128, 1024]
  y_blk = o @ Wo
32 independent blocks; 8 cores x 4 blocks, zero collectives.

v2 (causal-skip restructure): q^T/k^T/v^T are materialized in TRUE
pseudo-position order (m = 16a + r), so S^T [kpos, qpos] decomposes into
16 kpos-strips x 4 qpos-chunks of which only the block-lower-triangular
half is live (53% of tiles). Off-diagonal live tiles need no mask at
all; the 16 diagonal 128x128 tiles get a plain tril affine_select.
S/PV run in bf16 (same modeled PE rate as f32r, half the DVE/SBUF
traffic); the qkv and Wo gemms stay f32r with weights DMA'd via dtype
bitcast (zero cast traffic). q^T/k^T come from SP-issued DMA-transposes
(bf16) + one strided re-layout copy each on Pool; v^T via PE transposes
with strided m-order evictions, then per-strip back-transposes into
[kpos, c] form for PV.
"""
import sys

sys.path.insert(0, '/opt/trn_rl_repo')

import numpy as np

B, L, D = 2, 2048, 1024
H = 16              # heads == blocks per batch
RB = 128            # x rows per block
D3 = 3 * D
NR = 16             # r-groups (192-col chunks per row)
NB = 4              # blocks per core
NCORES = 8
P = 128
NCH = 6             # Wqkv 512-col streaming chunks
NC = 4              # qpos chunks of 512 per block
NT = 16             # kpos strips of 128 per block

_cached = {}


def _build_program():
    import concourse.bass as bass
    import concourse.mybir as mybir
    import concourse.tile as tile
    from concourse.tile import add_dep_helper

    f32 = mybir.dt.float32
    f32r = mybir.dt.float32r
    bf16 = mybir.dt.bfloat16
    EXP = mybir.ActivationFunctionType.Exp
    GE = mybir.AluOpType.is_ge

    nc = bass.Bass()
    xs = nc.declare_dram_parameter("xs", [NB, RB, D], f32, isOutput=False)
    wqkv = nc.declare_dram_parameter("wqkv", [D, D3], f32, isOutput=False)
    wo = nc.declare_dram_parameter("wo", [D, D], f32, isOutput=False)
    ys = nc.declare_dram_parameter("ys", [NB, RB, D], f32, isOutput=True)
    DBG = _cached.get("debug", False)
    if DBG:
        d_qkv = nc.declare_dram_parameter("d_qkv", [P, D3], f32, isOutput=True)
        d_qT = nc.declare_dram_parameter("d_qT", [64, L], f32, isOutput=True)
        d_kT = nc.declare_dram_parameter("d_kT", [64, L], f32, isOutput=True)
        d_vT = nc.declare_dram_parameter("d_vT", [64, L], f32, isOutput=True)
        d_va = nc.declare_dram_parameter("d_va", [P, NT * 65], f32, isOutput=True)
        d_pt = nc.declare_dram_parameter("d_pt", [P, 1024], f32, isOutput=True)
        d_osb = nc.declare_dram_parameter("d_osb", [64, 512], f32, isOutput=True)
        d_qkvT = nc.declare_dram_parameter("d_qkvT", [P, 3072], f32, isOutput=True)

    from contextlib import ExitStack
    with tile.TileContext(nc) as tc:
        with ExitStack() as _stk:
            def _pool(**kw):
                return _stk.enter_context(tc.tile_pool(**kw))

            constp = _pool(name="const", bufs=1)
            wqp = _pool(name="wq", bufs=2)
            wop = _pool(name="wop", bufs=1)
            xp = _pool(name="xp", bufs=2)
            xtp = _pool(name="xtp", bufs=1)
            qkvp = _pool(name="qkvp", bufs=1)
            qkcrap = _pool(name="qkcra", bufs=2)
            qtmp = _pool(name="qtm", bufs=2)
            ktmp = _pool(name="ktm", bufs=2)
            vtmp = _pool(name="vtm", bufs=2)
            vaugp = _pool(name="vaug", bufs=2)
            wlp = _pool(name="wl", bufs=2)
            ptp = _pool(name="pt", bufs=3)
            nrmp = _pool(name="nrm", bufs=2)
            yop = _pool(name="yo", bufs=2)
            qpsp = _pool(name="qps", bufs=2, space="PSUM")
            sttp = _pool(name="stt", bufs=2, space="PSUM")
            otqp = _pool(name="otq", bufs=2, space="PSUM")
            _pend_nops = []

            def absorb_on(eng, *prods):
                # Walrus caps every instruction at ONE sync wait. Emit
                # queue-local nops that sync-depend on each producer; the
                # post-pass elides waits covered by these earlier nops.
                for p in prods:
                    if p is None:
                        continue
                    n = eng.nop(hint="dep")
                    add_dep_helper(n.ins, p.ins, sync=True)
                    _pend_nops.append(n)

            def pin(h):
                # keep absorber nops scheduled before their instruction
                while _pend_nops:
                    n = _pend_nops.pop()
                    add_dep_helper(h.ins, n.ins, sync=False)
                return h

            dma_hs = []

            def dma(eng, dst, src, *deps):
                _pend_nops.clear()
                absorb_on(eng, *deps)
                h = pin(eng.dma_start(dst, src))
                dma_hs.append(h)
                return h

            # shared PSUM ring [128, 512] f32 for qkv-gemm outs, packed
            # transposes, bc broadcasts and wo-gemm outs
            qps_readers = []
            qps_n = [0]

            def qps_tile():
                n = qps_n[0]
                if n >= 2:
                    absorb_on(nc.tensor, qps_readers[n - 2])
                qps_n[0] += 1
                return qpsp.tile([P, 512], f32, tag="qps", name="qpstile")

            # ---- constants
            identb = constp.tile([P, 128], bf16, tag="identb")
            h_idm = nc.gpsimd.memset(identb[:], 0.0)
            absorb_on(nc.gpsimd, h_idm)
            h_idb = nc.gpsimd.affine_select(
                out=identb[:], in_=identb[:],
                compare_op=mybir.AluOpType.not_equal,
                fill=1.0, base=0, pattern=[[-1, 128]], channel_multiplier=1)
            identr_f = constp.tile([P, 128], f32, tag="identr")
            h_idm2 = nc.gpsimd.memset(identr_f[:], 0.0)
            absorb_on(nc.gpsimd, h_idm2)
            h_idr = nc.gpsimd.affine_select(
                out=identr_f[:], in_=identr_f[:],
                compare_op=mybir.AluOpType.not_equal,
                fill=1.0, base=0, pattern=[[-1, 128]], channel_multiplier=1)
            identr = identr_f[:]
            ones_f = constp.tile([1, 128], f32, tag="onesf")
            h_of = nc.gpsimd.memset(ones_f[:], 1.0)
            onesr = constp.tile([1, 128], f32r, tag="onesr")
            absorb_on(nc.vector, h_of)
            h_ones = nc.vector.tensor_copy(onesr[:], ones_f[:])

            # ---- phase A: x load + f32r transpose -> xT
            xT = xtp.tile([P, NB, 8, P], f32r, tag="xT")
            x_ev = []           # per block: last xT eviction handle (DVE)
            x_tr = []           # per block: last x transpose handle (PE)
            h_xev = None
            for i in range(NB):
                x_sb = xp.tile([P, D], f32, tag="x")
                h_x = dma(nc.gpsimd, x_sb[:], xs[i],
                          x_tr[i - 2] if i >= 2 else None)
                absorb_on(nc.tensor, h_x, h_idr if i == 0 else None)
                for g in range(2):
                    tp = qps_tile()
                    h_tr = None
                    for j in range(4):
                        h_tr = nc.tensor.transpose(
                            tp[:, 128 * j:128 * j + 128],
                            x_sb[:, 512 * g + 128 * j:512 * g + 128 * j + 128],
                            identr)
                    absorb_on(nc.vector, h_tr)
                    h_xev = nc.vector.tensor_copy(
                        xT[:, i, 4 * g:4 * g + 4, :],
                        tp[:].rearrange("p (k a) -> p k a", k=4))
                    qps_readers.append(h_xev)
                x_ev.append(h_xev)
                x_tr.append(h_tr)

            # ---- phase A: stream Wqkv (f32r bitcast, no casts), qkv gemms,
            # and per-block span DMA-transposes (3 per block, interleaved)
            qkv = qkvp.tile([P, NB, D3], bf16, tag="qkv")
            qkvT = qkvp.tile([P, NB, 24, P], bf16, tag="qkvT")
            evict_h = {}        # (nch, i) -> eviction handle
            span_h = {}         # (i, w) -> dma-transpose handle
            wq_hist = []
            def emit_spans(w):
                for i in range(NB):
                    _pend_nops.clear()
                    absorb_on(nc.sync, evict_h[(3 * w + 2, i)])
                    h_sp = pin(nc.sync.dma_start_transpose(
                        qkvT[:, i, 12 * w:12 * w + 12, :],
                        qkv[:, i, 1536 * w:1536 * w + 1536]))
                    dma_hs.append(h_sp)
                    span_h[(i, w)] = h_sp

            for nch in range(NCH):
                if nch == 4:
                    emit_spans(0)
                wq_t = wqp.tile([P, 8, 512], f32r, tag="wq")
                h_wd = dma(
                    nc.gpsimd, wq_t[:],
                    wqkv.rearrange("(kc p) n -> p kc n", p=P)
                    [:, :, 512 * nch:512 * nch + 512].bitcast(f32r),
                    wq_hist[-2] if len(wq_hist) >= 2 else None,
                    *span_h.values())
                h_mm = None
                for i in range(NB):
                    qp = qps_tile()
                    absorb_on(nc.tensor, h_wd if i == 0 else None,
                              x_ev[i] if nch == 0 else None)
                    for k in range(8):
                        h_mm = nc.tensor.matmul(
                            qp[:, 0:512],
                            xT[:, i, k, :],
                            wq_t[:, k, :],
                            start=(k == 0), stop=(k == 7),
                        )
                    absorb_on(nc.vector, h_mm)
                    h_ev = nc.vector.tensor_copy(
                        qkv[:, i, 512 * nch:512 * nch + 512], qp[:, 0:512])
                    qps_readers.append(h_ev)
                    evict_h[(nch, i)] = h_ev
                wq_hist.append(h_mm)
            emit_spans(1)

            # ---- phase A: Wo resident (f32r bitcast)
            wo_sb = wop.tile([P, 8, D], f32r, tag="wo")
            h_wo_dma = []
            for wc in range(2):
                h = dma(
                    nc.gpsimd,
                    wo_sb[:, :, 512 * wc:512 * wc + 512],
                    wo.rearrange("(kc p) n -> p kc n", p=P)
                    [:, :, 512 * wc:512 * wc + 512].bitcast(f32r),
                    h_wo_dma[0] if wc == 1 else None,
                    *[h for h in span_h.values()][-8:])
                h_wo_dma.append(h)

            # ---- per-block setup pieces (emitted interleaved)
            def strip_width(t, c):
                j = t - 4 * c
                return 512 if j < 0 else 512 - 128 * j

            blk = {}            # i -> dict of tiles/handles
            pt_ring = []        # (exp, last_pv, last_writer) per pt use
            pt_n = [0]

            # six m-order re-layout copies per block, from the 128-col slot
            # decomposition: global col g = 128*S + p; q/k/v of r-group r sit
            # at (partition base, slot offset mod 3) per r parity.
            #   name: (dst, src_pbase, slot_off, r0, engine)
            REL = [
                ('vT', 0, 1, 0), ('vT', 64, 2, 1),
                ('kT', 64, 0, 0), ('kT', 0, 2, 1),
                ('qT', 0, 0, 0), ('qT', 64, 1, 1),
            ]

            def war_pe():
                return pt_ring[-1][1] if pt_ring else None

            def emit_relayout(i, idx):
                b = blk.setdefault(i, {})
                if 'qT' not in b:
                    b['qT'] = qtmp.tile([64, L], bf16, tag="qT", name="qTm")
                    b['kT'] = ktmp.tile([64, L], bf16, tag="kT", name="kTm")
                    b['vT'] = vtmp.tile([64, L], bf16, tag="vT", name="vTm")
                    b['rel'] = {}
                name, pbase, off, r0 = REL[idx]
                eng = nc.gpsimd if name == 'kT' else nc.vector
                absorb_on(eng, span_h[(i, 0)], span_h[(i, 1)],
                          war_pe() if i >= 2 else None)
                h = eng.tensor_copy(
                    b[name][:].rearrange("p (a r) -> p a r", r=NR)
                    [:, :, r0:NR:2],
                    qkvT[pbase:pbase + 64, i, off:24:3, :]
                    .rearrange("p u a -> p a u"))
                b['rel'][(name, r0)] = h
                return h

            def emit_vaug(i, t_lo, t_hi):
                b = blk[i]
                if 'vaug' not in b:
                    b['vaug'] = vaugp.tile([P, NT, 65], bf16, tag="vaug", name="vaug")
                    # ones live only in col 64, which later blocks' evictions
                    # never touch: initialize the two pool slots once
                    if i < 2:
                        b['h_vm'] = nc.gpsimd.memset(b['vaug'][:], 1.0)
                    else:
                        b['h_vm'] = None
                    b['h_bt_ev'] = [None] * NT
                for t in range(t_lo, t_hi):
                    tp = qps_tile()
                    tpb = tp[:].bitcast(bf16)
                    absorb_on(nc.tensor,
                              b['rel'][('vT', 0)] if t == t_lo else None,
                              b['rel'][('vT', 1)] if t == t_lo else None)
                    h_bt = nc.tensor.transpose(
                        tpb[:, 0:64],
                        b['vT'][:, 128 * t:128 * t + 128],
                        identb[0:64, 0:64])
                    absorb_on(nc.vector, h_bt,
                              b['h_vm'] if t == t_lo else None)
                    h_be = nc.vector.tensor_copy(
                        b['vaug'][:, t, 0:64], tpb[:, 0:64])
                    qps_readers.append(h_be)
                    b['h_bt_ev'][t] = h_be
                    b['h_bt'] = h_bt

            # ---- attention per block, c-major with strip pairs
            stt_ring = []       # exp handle per stt tile use
            stt_n = [0]
            otq_readers = []    # per otq alloc: a normalization reader
            otq_n = [0]
            mul_hist = []
            y_hist = []
            pending_tail = []
            tail_last = {}

            def emit_tail(ti, t_wo_lhsT, t_h_mul):
                absorb_on(nc.tensor, t_h_mul,
                          h_wo_dma[1] if ti == 0 else None)
                y_sb = yop.tile([P, D], f32, tag="y")
                h_ye = None
                for n2 in range(2):
                    yp = qps_tile()
                    for k in range(8):
                        tail_last["womm"] = nc.tensor.matmul(
                            yp[:, 0:512],
                            t_wo_lhsT[:, k, :],
                            wo_sb[:, k, 512 * n2:512 * n2 + 512],
                            start=(k == 0), stop=(k == 7),
                        )
                    absorb_on(nc.vector, tail_last["womm"],
                              y_hist[-1][1] if (n2 == 0 and y_hist) else None)
                    h_ye = nc.vector.tensor_copy(
                        y_sb[:, 512 * n2:512 * n2 + 512], yp[:, 0:512])
                    tail_last["ye"] = h_ye
                    qps_readers.append(h_ye)
                h_yd = dma(nc.gpsimd, ys[ti], y_sb[:], h_ye,
                           *[h for h in span_h.values()][-2:])
                y_hist.append((h_ye, h_yd))

            # block 0 (and block 1 vT) setup before the attention loop
            for idx in range(6):
                emit_relayout(0, idx)
            emit_vaug(0, 0, NT)

            def tap(dst, src_ap, dep):
                np_ = src_ap.shape[0]
                nf = src_ap.free_size()
                flat_src = src_ap.rearrange(
                    " ".join(["p"] + [f"f{k}" for k in range(len(src_ap.shape) - 1)])
                    + " -> p (" + " ".join(f"f{k}" for k in range(len(src_ap.shape) - 1)) + ")")                     if len(src_ap.shape) > 2 else src_ap
                for o in range(0, nf, 1024):
                    wdt = min(1024, nf - o)
                    t_sb = yop.tile([P, D], f32, tag="y", name="tapbuf")
                    _pend_nops.clear()
                    absorb_on(nc.vector, dep, *dma_hs[-4:])
                    hh = pin(nc.vector.tensor_copy(
                        t_sb[0:np_, 0:wdt], flat_src[:, o:o + wdt]))
                    dma(nc.gpsimd, dst[:, o:o + wdt] if nf > 1024 else dst[0:np_, 0:nf],
                        t_sb[0:np_, 0:wdt], hh)

            h_exp = h_pv = h_mul = None
            for i in range(NB):
                b = blk[i]
                qT, kT, v_aug = b['qT'], b['kT'], b['vaug']
                h_bt_ev = b['h_bt_ev']
                wo_lhsT = wlp.tile([P, 8, P], f32r, tag="wl")
                if pending_tail:
                    emit_tail(*pending_tail.pop(0))
                absorb_on(nc.tensor, b['rel'][('qT', 0)],
                          b['rel'][('qT', 1)], b['rel'][('kT', 0)],
                          b['rel'][('kT', 1)])
                if DBG and i == 0:
                    tap(d_qkv, qkv[:, 0, :], evict_h[(5, 0)])
                    tap(d_qkvT, qkvT[:, 0, :, :].rearrange("p s a -> p (s a)"),
                        span_h[(0, 2)])
                    tap(d_qT, b['qT'][:], b['rel'][('qT', 1)])
                    tap(d_kT, b['kT'][:], b['rel'][('kT', 1)])
                    tap(d_vT, b['vT'][:], b['rel'][('vT', 1)])
                    tap(d_va, b['vaug'][:].rearrange("p t c -> p (t c)"),
                        b['h_bt_ev'][NT - 1])
                for c in range(NC):
                    # interleave next block's setup at chunk boundaries
                    if i + 1 < NB:
                        if c == 0:
                            emit_relayout(i + 1, 0)
                            emit_relayout(i + 1, 1)
                        elif c == 1:
                            emit_relayout(i + 1, 2)
                            emit_relayout(i + 1, 3)
                            emit_vaug(i + 1, 0, 8)
                        elif c == 2:
                            emit_relayout(i + 1, 4)
                            emit_relayout(i + 1, 5)
                            emit_vaug(i + 1, 8, NT)
                    cm0 = 512 * c
                    tmax = 4 * c + 3
                    if otq_n[0] >= 2:
                        absorb_on(nc.tensor, otq_readers[otq_n[0] - 2])
                    otq_n[0] += 1
                    otq = otqp.tile([65, 512], f32, tag="otq", name="otq")
                    for t0 in range(0, tmax + 1, 2):
                        pair = [t for t in (t0, t0 + 1) if t <= tmax]
                        widths = [strip_width(t, c) for t in pair]
                        # S matmuls for the pair into one stt tile
                        if stt_n[0] >= 2:
                            absorb_on(nc.tensor, stt_ring[stt_n[0] - 2])
                        stt_n[0] += 1
                        stt = sttp.tile([P, 1024], f32, tag="stt")
                        off = 0
                        offs = []
                        h_s = None
                        for t, w in zip(pair, widths):
                            absorb_on(nc.tensor,
                                      h_bt_ev[t] if c == t // 4 else None)
                            h_s = nc.tensor.matmul(
                                stt[:, off:off + w],
                                kT[:, 128 * t:128 * t + 128],
                                qT[:, cm0 + 512 - w:cm0 + 512],
                                start=True, stop=True,
                            )
                            offs.append(off)
                            off += w
                        # one exp over the pair
                        if pt_n[0] >= 3:
                            absorb_on(nc.scalar, pt_ring[pt_n[0] - 3][1],
                                      pt_ring[pt_n[0] - 3][2])
                        pt = ptp.tile([P, 1024], bf16, tag="pt")
                        pt_n[0] += 1
                        absorb_on(nc.scalar, h_s)
                        h_exp = nc.scalar.activation(
                            pt[:, 0:off], stt[:, 0:off], EXP, scale=0.25)
                        stt_ring.append(h_exp)
                        # masks on diagonal strips (t in [4c, 4c+3])
                        h_mask = {}
                        h_lastw = h_exp
                        for t, w, o in zip(pair, widths, offs):
                            if t >= 4 * c:
                                absorb_on(nc.gpsimd, h_exp)
                                h_mask[t] = nc.gpsimd.affine_select(
                                    out=pt[:, o:o + 128],
                                    in_=pt[:, o:o + 128],
                                    compare_op=GE, fill=0.0, base=0,
                                    pattern=[[1, 128]], channel_multiplier=-1)
                                h_lastw = h_mask[t]
                        # PV accumulation
                        for t, w, o in zip(pair, widths, offs):
                            absorb_on(nc.tensor, h_mask.get(t, h_exp))
                            h_pv = nc.tensor.matmul(
                                otq[:, 512 - w:512],
                                v_aug[:, t, :],
                                pt[:, o:o + w],
                                start=(t == 0), stop=(t == tmax),
                            )
                        pt_ring.append((h_exp, h_pv, h_lastw))
                        if DBG and i == 0 and c == 0 and t0 == 0:
                            tap(d_pt, pt[:, 0:off], h_lastw)
                    # normalization + wo_lhsT build for this chunk
                    rcp = nrmp.tile([1, 512], f32r, tag="rcp")
                    absorb_on(nc.vector, h_pv)
                    with nc.allow_low_precision(reason="f32r rounding of 1/d"):
                        h_rcp = nc.vector.reciprocal(rcp[:], otq[64:65, :])
                    o_sb = nrmp.tile([64, 512], bf16, tag="osb")
                    h_osb = nc.vector.tensor_copy(o_sb[:], otq[0:64, :])
                    otq_readers.append(h_osb)
                    if DBG and i == 0 and c == 0:
                        tap(d_osb, o_sb[:], h_osb)
                    bc = qps_tile()
                    absorb_on(nc.tensor, h_rcp)
                    h_bc = nc.tensor.matmul(
                        bc[:, 0:512], onesr[:], rcp[:, 0:512],
                        start=True, stop=True,
                    )
                    absorb_on(nc.vector, h_bc)
                    for par in range(2):
                        src = o_sb[:, par:512:2].rearrange(
                            "p (a ch) -> p ch a", ch=8)
                        bsr = bc[0:64, par:512:2].rearrange(
                            "p (a ch) -> p ch a", ch=8)
                        h_mul = nc.vector.tensor_mul(
                            wo_lhsT[64 * par:64 * par + 64, :,
                                    32 * c:32 * c + 32],
                            src, bsr)
                        mul_hist.append(h_mul)
                        qps_readers.append(h_mul)
                pending_tail.append((i, wo_lhsT, h_mul))

            while pending_tail:
                emit_tail(*pending_tail.pop(0))

            # absorb the kernel-tail drain's dependencies onto SP nops
            absorb_on(nc.sync, *dma_hs)
            absorb_on(nc.sync, tail_last["ye"], tail_last["womm"],
                      h_mul, h_pv, h_exp, h_ones)

    return nc


def _elide_covered_waits(nc):
    """Walrus rejects >1 sync wait per instruction. Each queue's sequencer
    processes waits in dispatch order, so a wait already issued earlier in
    the same queue gates every later instruction in that queue. Drop waits
    that an earlier same-queue instruction (incl. absorber nops) covers."""
    observed = {}   # engine -> {sem_id: max waited value}
    leftover = []
    for inst in nc.all_instructions():
        si = inst.sync_info
        if si is None:
            continue
        if type(inst).__name__ in ("InstEventSemaphore", "InstTrigger"):
            continue  # barrier-protocol handshakes, not data waits
        eng = str(inst.engine)
        short = eng.split(".")[-1]
        obs = observed.setdefault(eng, {})
        ow = list(si.on_wait or [])
        keep = []
        for w in ow:
            if obs.get(w.id, -1) >= w.wait_value:
                continue
            if w.ant_name.startswith(short + "_"):
                # wait on this engine's own completion counter: satisfied
                # by in-order execution of the same queue
                obs[w.id] = max(obs.get(w.id, -1), w.wait_value)
                continue
            keep.append(w)
            obs[w.id] = max(obs.get(w.id, -1), w.wait_value)
        if len(keep) != len(ow):
            si.on_wait = keep
            inst.sync_info = si
        if len(keep) > 1:
            leftover.append((inst.name, type(inst).__name__, eng,
                             [(w.ant_name, w.wait_value) for w in keep]))
    if leftover:
        import logging
        logging.warning("multi-wait instructions remain: %s", leftover[:12])


def _get_program():
    if "nc" not in _cached:
        nc = _build_program()
        _elide_covered_waits(nc)
        _cached["nc"] = nc
    return _cached["nc"]


def kernel(x=None, mask=None, Wqkv=None, Wo=None, **_ignored):
    """Full inputs -> full output. mask is ignored (guaranteed causal tril)."""
    from concourse.bass_utils import run_bass_kernel_spmd

    x = np.ascontiguousarray(np.asarray(x, dtype=np.float32))
    Wqkv = np.ascontiguousarray(np.asarray(Wqkv, dtype=np.float32))
    Wo = np.ascontiguousarray(np.asarray(Wo, dtype=np.float32))

    nc = _get_program()
    in_maps = []
    for c in range(NCORES):
        shards = []
        for g in range(NB * c, NB * c + NB):
            b, h = divmod(g, H)
            shards.append(x[b, RB * h:RB * h + RB, :])
        in_maps.append({
            "xs": np.ascontiguousarray(np.stack(shards, axis=0)),
            "wqkv": Wqkv,
            "wo": Wo,
        })

    res = run_bass_kernel_spmd(nc, in_maps, core_ids=list(range(NCORES)))
    y = np.empty((B, L, D), dtype=np.float32)
    for c in range(NCORES):
        ysc = res.results[c]["ys"]
        for idx, g in enumerate(range(NB * c, NB * c + NB)):
            b, h = divmod(g, H)
            y[b, RB * h:RB * h + RB, :] = ysc[idx]
    return y
